# revision 65
# baseline (speedup 1.0000x reference)
"""MultiHeadAttention TRN2 kernel: tensor-parallel over heads across 8 NeuronCores.

Problem (hardcoded): BS=2, QLEN=2048, DIM=1024, NHEADS=16, HEAD=64.
  q = split_heads(x @ q_w.T + q_b) / sqrt(64)
  s = q @ k.T + mask ; w = softmax(s) ; ctx = w @ v
  out = merge_heads(ctx) @ o_w.T + o_b

Sharding: core c computes heads {2c, 2c+1} (rows 128c:128c+128 of q/k/v weights,
cols 128c:128c+128 of o_w).  Each core emits a full-shape partial of the output
projection; the host sums the 8 partials and adds o_b (row-parallel gather).

Device layout (all "T" tensors are [feature, token] so the contraction dim of
every matmul sits on SBUF partitions):
  QT/KT/VT [128, 4096]  = projections for 2 local heads (partitions = head dims)
  scores^T tiles [128 kpos, q]; exp() on ScalarE with the attention mask fed
  through the activation bias port (mask varies along k = partitions).
  PV stationary operands are V tiles [128 kpos, 65] with an extra all-ones
  column, so each PV accumulation also produces the softmax denominators for
  free: each head's ctx lands on psum partitions 0..63 with its sums row on
  partition 64 (separate psum tile per head).
  Denominators: copy the two sums rows to SBUF (bf16), broadcast each head's
  sums to its 64 partitions with two accumulated [1,128]-stationary matmuls,
  reciprocal_approx_fast on DVE, then two elementwise multiplies produce
  normalized ctx^T in SBUF (partition-base-64 operands align head 1).

Scheduling notes:
  - Each qtile's attention is ONE 32-unit stream across both heads; the PV
    matmuls trail their exp by 3 units (software pipeline), so the PE stays
    busy while ScalarE exps, and the lag crosses the head boundary with no
    drain bubble.  PE p-state only reaches max after ~3us continuous work,
    so every bubble costs double.
  - Qtile i's divide chain + output projection are emitted INSIDE qtile
    i+1's stream (divide at unit 1; outproj chunks injected every 4th
    unit), so they fill exp-wait bubbles instead of serializing.
  - The first attention units are interleaved with the last QKV groups
    (generator-driven emission) so the PE carries its max p-state across
    the phase boundary; steady-state units then run at ~1.09us, just
    under the 1.11us exp duration -- the kernel is ScalarE/exp-bound.
  - V transposes: the DMA xbar intermittently corrupts tiles when transpose
    transfers from different queues overlap, so all transposes go through
    the Sync queue alone, one [128,128] both-heads transpose per kt tile
    into a staging tile; idle GpSimd splits heads into the v65 layout
    (SBUF->SBUF, allowed on Pool).  x loads ride the ScalarE queue (idle
    until attention), out writes ride Sync after transposes are done.
  - PSUM budget is exactly 8 banks: work pool 2x[128,1024]f32 (scores /
    sums-broadcast / outproj) + acc pool 2x[128,1024]f32 (PV accumulators;
    QKV-phase psv rides the same slots).
  - Output partials are written bf16 (halves eviction + DMA cost; the host
    sums 8 partials in f32, adding ~0.1% error vs the 2e-2 budget).
"""

import sys

if "/opt/trn_rl_repo" not in sys.path:
    sys.path.insert(0, "/opt/trn_rl_repo")

import math
from contextlib import ExitStack

import ml_dtypes
import numpy as np

import concourse.bass as bass
import concourse.tile as tile
from concourse import bacc, mybir
from concourse.bass_utils import run_bass_kernel_spmd


# ---- problem constants ----
BS, QLEN, DIM, NHEADS = 2, 2048, 1024, 16
HEAD = DIM // NHEADS            # 64
NTOK = BS * QLEN                # 4096
NCORES = 8
HPC = NHEADS // NCORES          # 2 heads per core
LDIM = HPC * HEAD               # 128 local dims per core
NKCH = DIM // 128               # 8 contraction chunks for projections
NTT = NTOK // 512               # 8 token tiles of 512 for projections
NKT = QLEN // 128               # 16 key tiles per batch
QTW = 1024                      # query tile width for attention
NQT = QLEN // QTW               # 2 query tiles per batch

DT = mybir.dt.bfloat16          # matmul compute dtype
NPDT = ml_dtypes.bfloat16
F32 = mybir.dt.float32

_cache = {}


def build_program(dump=False):
    """Build + compile the single-core SPMD Bass program."""
    nc = bacc.Bacc("TRN2", target_bir_lowering=False, debug=False,
                   num_devices=NCORES)
    dbg = {}
    if dump:
        for nm, shp, dt_ in (("d_qt", [128, NTOK], DT),
                             ("d_kt", [128, NTOK], DT),
                             ("d_vt", [128, NTOK], DT),
                             ("d_v65", [128, BS * 2 * NKT * 65], DT),
                             ("d_ct", [128, NTOK], DT),
                             ("d_rc", [128, QTW], F32)):
            dbg[nm] = nc.dram_tensor(nm, shp, dt_,
                                     kind="ExternalOutput").ap()

    xt = nc.dram_tensor("xt", [DIM, NTOK], DT, kind="ExternalInput").ap()
    wq = nc.dram_tensor("wq", [DIM, LDIM], DT, kind="ExternalInput").ap()
    wk = nc.dram_tensor("wk", [DIM, LDIM], DT, kind="ExternalInput").ap()
    wv = nc.dram_tensor("wv", [DIM, LDIM], DT, kind="ExternalInput").ap()
    wo = nc.dram_tensor("wo", [LDIM, DIM], DT, kind="ExternalInput").ap()
    qb = nc.dram_tensor("qb", [LDIM, 1], F32, kind="ExternalInput").ap()
    kb = nc.dram_tensor("kb", [LDIM, 1], F32, kind="ExternalInput").ap()
    vb = nc.dram_tensor("vb", [LDIM, 1], F32, kind="ExternalInput").ap()
    maskd = nc.dram_tensor("maskd", [128, BS * NKT], F32,
                           kind="ExternalInput").ap()
    out = nc.dram_tensor("out", [NTOK, DIM], DT, kind="ExternalOutput").ap()

    with tile.TileContext(nc) as tc, ExitStack() as ctx:
        singles = ctx.enter_context(tc.tile_pool(name="singles", bufs=1))
        evict = ctx.enter_context(tc.tile_pool(name="evict", bufs=2))
        stp = ctx.enter_context(tc.tile_pool(name="stp", bufs=5))
        srp = ctx.enter_context(tc.tile_pool(name="srp", bufs=2))
        tsp = ctx.enter_context(tc.tile_pool(name="tsp", bufs=2))
        work = ctx.enter_context(
            tc.tile_pool(name="work", bufs=2, space="PSUM"))
        accp = ctx.enter_context(
            tc.tile_pool(name="accp", bufs=2, space="PSUM"))

        # --- resident SBUF tensors ---
        wq_sb = singles.tile([128, NKCH, LDIM], DT, tag="wq")
        wk_sb = singles.tile([128, NKCH, LDIM], DT, tag="wk")
        wv_sb = singles.tile([128, NKCH, LDIM], DT, tag="wv")
        wo_sb = singles.tile([LDIM, DIM], DT, tag="wo")
        qb_sb = singles.tile([LDIM, 1], F32, tag="qb")
        kb_sb = singles.tile([LDIM, 1], F32, tag="kb")
        vb_sb = singles.tile([LDIM, 1], F32, tag="vb")
        mask_sb = singles.tile([128, BS * NKT], F32, tag="mask")
        onesA_sb = singles.tile([1, 128], DT, tag="onesA")
        onesB_sb = singles.tile([1, 128], DT, tag="onesB")
        qt_sb = singles.tile([128, NTOK], DT, tag="qt")
        kt_sb = singles.tile([128, NTOK], DT, tag="kt")
        vt_sb = singles.tile([128, NTOK], DT, tag="vt")
        ct_sb = singles.tile([128, NTOK], DT, tag="ct")
        rc_sb = singles.tile([128, QTW], F32, tag="rc")
        # V tiles per (batch, kt, head): [128 kpos, 64 dims | ones col]
        v65_sb = singles.tile([128, BS, 2 * NKT, 128], DT, tag="v65")
        # full x^T resident in SBUF, loaded one 512-token group per DMA
        xt_sb = singles.tile([128, NKCH, NTOK], DT, tag="xts")
        xt_r = xt.rearrange("(c p) t -> p c t", p=128)

        # load order: first QKV weights + the first x group (the critical
        # path to the first matmul), then everything needed later.
        for w_sb, w_dram in ((wq_sb, wq), (wk_sb, wk), (wv_sb, wv)):
            nc.sync.dma_start(
                w_sb[:], w_dram.rearrange("(c p) m -> p c m", p=128))
        nc.sync.dma_start(qb_sb[:], qb[:])
        nc.sync.dma_start(kb_sb[:], kb[:])
        nc.sync.dma_start(vb_sb[:], vb[:])
        nc.scalar.dma_start(mask_sb[:], maskd[:])
        nc.scalar.dma_start(wo_sb[:], wo[:])
        nc.vector.memset(v65_sb[:, :, :, 64:65], 1.0)
        nc.vector.memset(onesA_sb[:], 0.0)
        nc.vector.memset(onesA_sb[0:1, 0:64], 1.0)
        nc.vector.memset(onesB_sb[:], 0.0)
        nc.vector.memset(onesB_sb[0:1, 64:128], 1.0)

        # --- phase 1: QKV projections, token-tile at a time ---
        nc.scalar.dma_start(xt_sb[:, :, 0:512], xt_r[:, :, 0:512])

        def emit_group(g):
            if g + 1 < NTT:
                gn = slice(512 * (g + 1), 512 * (g + 2))
                nc.scalar.dma_start(xt_sb[:, :, gn], xt_r[:, :, gn])
            psqk = work.tile([128, 1024], F32, tag="work")
            psv = accp.tile([128, 1024], F32, tag="acc")
            for c in range(NKCH):
                xt_t = xt_sb[:, c, 512 * g:512 * (g + 1)]
                st_, sp_ = (c == 0), (c == NKCH - 1)
                nc.tensor.matmul(psqk[:, 0:512], wq_sb[:, c, :], xt_t,
                                 start=st_, stop=sp_)
                nc.tensor.matmul(psqk[:, 512:1024], wk_sb[:, c, :], xt_t,
                                 start=st_, stop=sp_)
                nc.tensor.matmul(psv[:, 0:512], wv_sb[:, c, :], xt_t,
                                 start=st_, stop=sp_)
            gs = slice(512 * g, 512 * (g + 1))
            nc.vector.tensor_scalar_add(qt_sb[:, gs], psqk[:, 0:512],
                                        qb_sb[:, 0:1])
            nc.vector.tensor_scalar_add(kt_sb[:, gs], psqk[:, 512:1024],
                                        kb_sb[:, 0:1])
            nc.vector.tensor_scalar_add(vt_sb[:, gs], psv[:, 0:512],
                                        vb_sb[:, 0:1])

            # transpose this group's V token blocks ([128 dims, 128 kpos]
            # -> [128 kpos, 128 dims], both heads at once) via DMA xbar, ALL
            # on the Sync queue: xbar transfers from different queues
            # intermittently corrupt each other, so they must be serialized
            # on a single queue.  A DVE copy then splits the two heads into
            # the ones-adjacent v65 layout.
            b = g // (NTT // BS)
            tstage = tsp.tile([128, 4, 128], DT, tag="ts", name="tstage")
            for j in range(4):
                ktl = 4 * (g % 4) + j
                t0 = QLEN * b + 128 * ktl
                nc.sync.dma_start(tstage[:, j, :], vt_sb[:, t0:t0 + 128],
                                  transpose=True)
            i0 = (4 * (g % 4)) * 2
            for h in range(2):
                nc.gpsimd.tensor_copy(
                    v65_sb[:, b, i0 + h:i0 + 8:2, 0:64],
                    tstage[:, :, 64 * h:64 * (h + 1)])

        # --- phase 2: attention + output projection per (batch, qtile) ---
        # Deferred-work queue: items emitted one per kt unit inside the next
        # qtile's loop so the divide / output projection / DMA issue work of
        # qtile i overlaps qtile i+1's scores+PV stream.
        from collections import deque
        pend_pe = deque()    # PE-work items (outproj): pop every other unit

        def emit_divide(cts, qs):
            """Denominators -> reciprocal -> normalized ctx in ct_sb.
            The sums-row copies run on ScalarE, which is naturally starved at
            qtile transitions; this keeps the DVE free to run the reciprocal
            as soon as the broadcast lands, releasing the work-pool slot."""
            srow0 = srp.tile([1, QTW], DT, tag="srow", name="srow0")
            srow1 = srp.tile([1, QTW], DT, tag="srow", name="srow1")
            nc.vector.tensor_copy(srow0[:], cts[0][64:65, :])
            nc.vector.tensor_copy(srow1[:], cts[1][64:65, :])
            sums_bc = work.tile([128, QTW], F32, tag="work", name="sums_bc")
            for j2 in range(2):
                js = slice(512 * j2, 512 * (j2 + 1))
                nc.tensor.matmul(sums_bc[:, js], onesA_sb[:],
                                 srow0[:, js], start=True, stop=False,
                                 skip_group_check=True)
                nc.tensor.matmul(sums_bc[:, js], onesB_sb[:],
                                 srow1[:, js], start=False, stop=True,
                                 skip_group_check=True)
            nc.vector.reciprocal_approx_fast(rc_sb[:], sums_bc[:])
            nc.vector.tensor_mul(ct_sb[0:64, qs], cts[0][0:64, :],
                                 rc_sb[0:64, :])
            nc.vector.tensor_mul(ct_sb[64:128, qs], cts[1][0:64, :],
                                 rc_sb[64:128, :])

        def defer_outproj(qs, t, evict_eng=None):
            def go():
                tok0 = qs.start + 128 * t
                o_ps = work.tile([128, 1024], F32, tag="work", name="o_ps")
                lhs = ct_sb[:, tok0:tok0 + 128]
                nc.tensor.matmul(o_ps[:, 0:512], lhs, wo_sb[:, 0:512],
                                 start=True, stop=True)
                nc.tensor.matmul(o_ps[:, 512:1024], lhs,
                                 wo_sb[:, 512:1024], start=True, stop=True)
                o_sb = evict.tile([128, 1024], DT, tag="osb", name="o_sb")
                if evict_eng is nc.scalar:
                    nc.scalar.activation(o_sb[:], o_ps[:],
                                         mybir.ActivationFunctionType.Copy)
                else:
                    nc.vector.tensor_copy(o_sb[:], o_ps[:])
                nc.sync.dma_start(out[tok0:tok0 + 128, :], o_sb[:])
            return go

        def attention_stream():
            """Yields once per attention unit so the caller can interleave
            the first few units with the tail of the QKV phase (carrying the
            PE p-state across the phase boundary).  PV accumulators are
            allocated lazily at the first PV so the accp ring stays free for
            the remaining QKV psv tiles."""
            prev_q = None
            for b in range(BS):
                for qt_i in range(NQT):
                    qs = slice(QLEN * b + QTW * qt_i,
                               QLEN * b + QTW * (qt_i + 1))
                    cts = []

                    def emit_pv(h, kt_i, st_t, cts=cts, b=b):
                        vsl = v65_sb[:, b, kt_i * 2 + h, 0:65]
                        st0, sp0 = (kt_i == 0), (kt_i == NKT - 1)
                        for j2 in range(2):
                            nc.tensor.matmul(
                                cts[h][0:65, 512 * j2:512 * (j2 + 1)],
                                vsl, st_t[:, 512 * j2:512 * (j2 + 1)],
                                start=st0, stop=sp0, skip_group_check=True)

                    # one continuous 32-unit stream across both heads: the
                    # lagged-PV pipeline crosses the head boundary with no
                    # drain gap.
                    lagged = []
                    for u in range(2 * NKT):
                        h, kt_i = u // NKT, u % NKT
                        hs = slice(HEAD * h, HEAD * (h + 1))
                        ks = slice(QLEN * b + 128 * kt_i,
                                   QLEN * b + 128 * (kt_i + 1))
                        s_ps = work.tile([128, QTW], F32, tag="work",
                                         name="s_ps")
                        for j2 in range(2):
                            qsub = slice(qs.start + 512 * j2,
                                         qs.start + 512 * (j2 + 1))
                            nc.tensor.matmul(
                                s_ps[:, 512 * j2:512 * (j2 + 1)],
                                kt_sb[hs, ks], qt_sb[hs, qsub],
                                start=True, stop=True)
                        st_t = stp.tile([128, QTW], DT, tag="st",
                                        name="st_t")
                        m_ap = mask_sb[:, b * NKT + kt_i:
                                       b * NKT + kt_i + 1]
                        nc.scalar.activation(
                            st_t[:], s_ps[:],
                            mybir.ActivationFunctionType.Exp, bias=m_ap)
                        lagged.append((h, kt_i, st_t))
                        # previous qtile's divide goes early so its outproj
                        # (and this qtile's first PV, via accp) unblock
                        if u == 3 and prev_q is not None:
                            emit_divide(*prev_q)
                            for t in range(QTW // 128):
                                pend_pe.append(defer_outproj(prev_q[1], t))
                            prev_q = None
                        if len(lagged) > 3:
                            if not cts:
                                cts.append(accp.tile([128, QTW], F32,
                                                     tag="acc", name="ct0"))
                                cts.append(accp.tile([128, QTW], F32,
                                                     tag="acc", name="ct1"))
                            emit_pv(*lagged.pop(0))
                        if u >= 5 and u % 4 == 1 and pend_pe:
                            pend_pe.popleft()()
                        yield
                    for item in lagged:
                        emit_pv(*item)
                    prev_q = (cts, qs)

            # final qtile drain (ScalarE is idle now: alternate evictors)
            emit_divide(*prev_q)
            for t in range(QTW // 128):
                defer_outproj(prev_q[1], t,
                              evict_eng=nc.scalar if t % 2 else nc.vector)()
            while pend_pe:
                pend_pe.popleft()()

        stream = attention_stream()
        for g in range(NTT):
            emit_group(g)
            # bridge the phase boundary: the first batch's attention units
            # (scores+exp only, no PV/psum pressure) slot between the last
            # QKV groups so the PE never idles at the transition.
            if 4 <= g < NTT - 1:
                next(stream, None)
        for _ in stream:
            pass

        if dump:
            nc.sync.dma_start(dbg["d_qt"][:], qt_sb[:])
            nc.sync.dma_start(dbg["d_kt"][:], kt_sb[:])
            nc.sync.dma_start(dbg["d_vt"][:], vt_sb[:])
            nc.sync.dma_start(
                dbg["d_v65"].rearrange("p (i c) -> p i c", c=65),
                v65_sb[:, :, :, 0:65].rearrange("p a b c -> p (a b) c"))
            nc.sync.dma_start(dbg["d_ct"][:], ct_sb[:])
            nc.sync.dma_start(dbg["d_rc"][:], rc_sb[:])

    nc.compile()
    return nc


def shard_inputs(input, mask, q_w, q_b, k_w, k_b, v_w, v_b, o_w, o_b):
    x = np.asarray(input, np.float32)
    xt = np.ascontiguousarray(x.T).astype(NPDT)
    m = np.asarray(mask, np.float32).reshape(BS, NKT, 128)
    maskd = np.ascontiguousarray(m.transpose(2, 0, 1).reshape(128, BS * NKT))
    scale = 1.0 / math.sqrt(HEAD)
    in_maps = []
    for c in range(NCORES):
        L = slice(LDIM * c, LDIM * (c + 1))
        in_maps.append({
            "xt": xt,
            "wq": np.ascontiguousarray((q_w[L, :] * scale).T).astype(NPDT),
            "wk": np.ascontiguousarray(k_w[L, :].T).astype(NPDT),
            "wv": np.ascontiguousarray(v_w[L, :].T).astype(NPDT),
            "wo": np.ascontiguousarray(o_w[:, L].T).astype(NPDT),
            "qb": (q_b[L] * scale).astype(np.float32).reshape(LDIM, 1),
            "kb": k_b[L].astype(np.float32).reshape(LDIM, 1),
            "vb": v_b[L].astype(np.float32).reshape(LDIM, 1),
            "maskd": maskd,
        })
    return in_maps


def run(in_maps, **kw):
    if "nc" not in _cache:
        _cache["nc"] = build_program()
    return run_bass_kernel_spmd(_cache["nc"], in_maps,
                                core_ids=list(range(NCORES)), **kw)


def kernel(input, mask, q_w, q_b, k_w, k_b, v_w, v_b, o_w, o_b,
           bs=BS, qlen=QLEN):
    assert int(bs) == BS and int(qlen) == QLEN
    in_maps = shard_inputs(np.asarray(input), np.asarray(mask),
                           np.asarray(q_w), np.asarray(q_b),
                           np.asarray(k_w), np.asarray(k_b),
                           np.asarray(v_w), np.asarray(v_b),
                           np.asarray(o_w), np.asarray(o_b))
    res = run(in_maps)
    acc = np.zeros((NTOK, DIM), np.float32)
    for r in res.results:
        acc += np.asarray(r["out"], np.float32)
    acc += np.asarray(o_b, np.float32)[None, :]
    return acc


# revision 66
# speedup vs baseline: 1.0375x; 1.0375x over previous
"""MultiHeadAttention TRN2 kernel: tensor-parallel over heads across 8 NeuronCores.

Problem (hardcoded): BS=2, QLEN=2048, DIM=1024, NHEADS=16, HEAD=64.
  q = split_heads(x @ q_w.T + q_b) / sqrt(64)
  s = q @ k.T + mask ; w = softmax(s) ; ctx = w @ v
  out = merge_heads(ctx) @ o_w.T + o_b

Sharding: core c computes heads {2c, 2c+1} (rows 128c:128c+128 of q/k/v weights,
cols 128c:128c+128 of o_w).  Each core emits a full-shape partial of the output
projection; the host sums the 8 partials and adds o_b (row-parallel gather).

Device layout (all "T" tensors are [feature, token] so the contraction dim of
every matmul sits on SBUF partitions):
  QT/KT/VT [128, 4096]  = projections for 2 local heads (partitions = head dims)
  scores^T tiles [128 kpos, q]; exp() on ScalarE with the attention mask fed
  through the activation bias port (mask varies along k = partitions).
  PV stationary operands are V tiles [128 kpos, 65] with an extra all-ones
  column, so each PV accumulation also produces the softmax denominators for
  free: each head's ctx lands on psum partitions 0..63 with its sums row on
  partition 64 (separate psum tile per head).
  Denominators: copy the two sums rows to SBUF (bf16), broadcast each head's
  sums to its 64 partitions with two accumulated [1,128]-stationary matmuls,
  reciprocal_approx_fast on DVE, then two elementwise multiplies produce
  normalized ctx^T in SBUF (partition-base-64 operands align head 1).

Scheduling notes:
  - Each qtile's attention is ONE 32-unit stream across both heads; the PV
    matmuls trail their exp by 3 units (software pipeline), so the PE stays
    busy while ScalarE exps, and the lag crosses the head boundary with no
    drain bubble.  PE p-state only reaches max after ~3us continuous work,
    so every bubble costs double.
  - Qtile i's divide chain + output projection are emitted INSIDE qtile
    i+1's stream (divide at unit 1; outproj chunks injected every 4th
    unit), so they fill exp-wait bubbles instead of serializing.
  - The first attention units are interleaved with the last QKV groups
    (generator-driven emission) so the PE carries its max p-state across
    the phase boundary; steady-state units then run at ~1.09us, just
    under the 1.11us exp duration -- the kernel is ScalarE/exp-bound.
  - V transposes: the DMA xbar intermittently corrupts tiles when transpose
    transfers from different queues overlap, so all transposes go through
    the Sync queue alone, one [128,128] both-heads transpose per kt tile
    into a staging tile; idle GpSimd splits heads into the v65 layout
    (SBUF->SBUF, allowed on Pool).  x loads ride the ScalarE queue (idle
    until attention), out writes ride Sync after transposes are done.
  - PSUM budget is exactly 8 banks: work pool 2x[128,1024]f32 (scores /
    sums-broadcast / outproj) + acc pool 2x[128,1024]f32 (PV accumulators;
    QKV-phase psv rides the same slots).
  - Output partials are written bf16 (halves eviction + DMA cost; the host
    sums 8 partials in f32, adding ~0.1% error vs the 2e-2 budget).
"""

import sys

if "/opt/trn_rl_repo" not in sys.path:
    sys.path.insert(0, "/opt/trn_rl_repo")

import math
from contextlib import ExitStack

import ml_dtypes
import numpy as np

import concourse.bass as bass
import concourse.tile as tile
from concourse import bacc, mybir
from concourse.bass_utils import run_bass_kernel_spmd


# ---- problem constants ----
BS, QLEN, DIM, NHEADS = 2, 2048, 1024, 16
HEAD = DIM // NHEADS            # 64
NTOK = BS * QLEN                # 4096
NCORES = 8
HPC = NHEADS // NCORES          # 2 heads per core
LDIM = HPC * HEAD               # 128 local dims per core
NKCH = DIM // 128               # 8 contraction chunks for projections
NTT = NTOK // 512               # 8 token tiles of 512 for projections
NKT = QLEN // 128               # 16 key tiles per batch
QTW = 1024                      # query tile width for attention
NQT = QLEN // QTW               # 2 query tiles per batch

DT = mybir.dt.bfloat16          # matmul compute dtype
NPDT = ml_dtypes.bfloat16
F32 = mybir.dt.float32

_cache = {}


def build_program(dump=False):
    """Build + compile the single-core SPMD Bass program."""
    nc = bacc.Bacc("TRN2", target_bir_lowering=False, debug=False,
                   num_devices=NCORES)
    dbg = {}
    if dump:
        for nm, shp, dt_ in (("d_qt", [128, NTOK], DT),
                             ("d_kt", [128, NTOK], DT),
                             ("d_vt", [128, NTOK], DT),
                             ("d_v65", [128, BS * 2 * NKT * 65], DT),
                             ("d_ct", [128, NTOK], DT),
                             ("d_rc", [128, QTW], F32)):
            dbg[nm] = nc.dram_tensor(nm, shp, dt_,
                                     kind="ExternalOutput").ap()

    xt = nc.dram_tensor("xt", [DIM, NTOK], DT, kind="ExternalInput").ap()
    wq = nc.dram_tensor("wq", [DIM, LDIM], DT, kind="ExternalInput").ap()
    wk = nc.dram_tensor("wk", [DIM, LDIM], DT, kind="ExternalInput").ap()
    wv = nc.dram_tensor("wv", [DIM, LDIM], DT, kind="ExternalInput").ap()
    wo = nc.dram_tensor("wo", [LDIM, DIM], DT, kind="ExternalInput").ap()
    qb = nc.dram_tensor("qb", [LDIM, 1], F32, kind="ExternalInput").ap()
    kb = nc.dram_tensor("kb", [LDIM, 1], F32, kind="ExternalInput").ap()
    vb = nc.dram_tensor("vb", [LDIM, 1], F32, kind="ExternalInput").ap()
    maskd = nc.dram_tensor("maskd", [128, BS * NKT], F32,
                           kind="ExternalInput").ap()
    out = nc.dram_tensor("out", [NTOK, DIM], DT, kind="ExternalOutput").ap()

    with tile.TileContext(nc) as tc, ExitStack() as ctx:
        singles = ctx.enter_context(tc.tile_pool(name="singles", bufs=1))
        evict = ctx.enter_context(tc.tile_pool(name="evict", bufs=2))
        stp = ctx.enter_context(tc.tile_pool(name="stp", bufs=5))
        srp = ctx.enter_context(tc.tile_pool(name="srp", bufs=2))
        tsp = ctx.enter_context(tc.tile_pool(name="tsp", bufs=2))
        work = ctx.enter_context(
            tc.tile_pool(name="work", bufs=2, space="PSUM"))
        accp = ctx.enter_context(
            tc.tile_pool(name="accp", bufs=2, space="PSUM"))

        # --- resident SBUF tensors ---
        wq_sb = singles.tile([128, NKCH, LDIM], DT, tag="wq")
        wk_sb = singles.tile([128, NKCH, LDIM], DT, tag="wk")
        wv_sb = singles.tile([128, NKCH, LDIM], DT, tag="wv")
        wo_sb = singles.tile([LDIM, DIM], DT, tag="wo")
        qb_sb = singles.tile([LDIM, 1], F32, tag="qb")
        kb_sb = singles.tile([LDIM, 1], F32, tag="kb")
        vb_sb = singles.tile([LDIM, 1], F32, tag="vb")
        mask_sb = singles.tile([128, BS * NKT], F32, tag="mask")
        onesA_sb = singles.tile([1, 128], DT, tag="onesA")
        onesB_sb = singles.tile([1, 128], DT, tag="onesB")
        qt_sb = singles.tile([128, NTOK], DT, tag="qt")
        kt_sb = singles.tile([128, NTOK], DT, tag="kt")
        vt_sb = singles.tile([128, NTOK], DT, tag="vt")
        ct_sb = singles.tile([128, NTOK], DT, tag="ct")
        rc_sb = singles.tile([128, QTW], F32, tag="rc")
        # V tiles per (batch, kt, head): [128 kpos, 64 dims | ones col]
        v65_sb = singles.tile([128, BS, 2 * NKT, 128], DT, tag="v65")
        # full x^T resident in SBUF, loaded one 512-token group per DMA
        xt_sb = singles.tile([128, NKCH, NTOK], DT, tag="xts")
        xt_r = xt.rearrange("(c p) t -> p c t", p=128)

        # load order: first QKV weights + the first x group (the critical
        # path to the first matmul), then everything needed later.
        for w_sb, w_dram in ((wq_sb, wq), (wk_sb, wk), (wv_sb, wv)):
            nc.sync.dma_start(
                w_sb[:], w_dram.rearrange("(c p) m -> p c m", p=128))
        nc.sync.dma_start(qb_sb[:], qb[:])
        nc.sync.dma_start(kb_sb[:], kb[:])
        nc.sync.dma_start(vb_sb[:], vb[:])
        nc.scalar.dma_start(mask_sb[:], maskd[:])
        nc.scalar.dma_start(wo_sb[:], wo[:])
        nc.vector.memset(v65_sb[:, :, :, 64:65], 1.0)
        nc.vector.memset(onesA_sb[:], 0.0)
        nc.vector.memset(onesA_sb[0:1, 0:64], 1.0)
        nc.vector.memset(onesB_sb[:], 0.0)
        nc.vector.memset(onesB_sb[0:1, 64:128], 1.0)

        # --- phase 1: QKV projections, token-tile at a time ---
        nc.scalar.dma_start(xt_sb[:, :, 0:512], xt_r[:, :, 0:512])

        def emit_group(g):
            if g + 1 < NTT:
                gn = slice(512 * (g + 1), 512 * (g + 2))
                nc.scalar.dma_start(xt_sb[:, :, gn], xt_r[:, :, gn])
            psqk = work.tile([128, 1024], F32, tag="work")
            psv = accp.tile([128, 1024], F32, tag="acc")
            for c in range(NKCH):
                xt_t = xt_sb[:, c, 512 * g:512 * (g + 1)]
                st_, sp_ = (c == 0), (c == NKCH - 1)
                nc.tensor.matmul(psqk[:, 0:512], wq_sb[:, c, :], xt_t,
                                 start=st_, stop=sp_)
                nc.tensor.matmul(psqk[:, 512:1024], wk_sb[:, c, :], xt_t,
                                 start=st_, stop=sp_)
                nc.tensor.matmul(psv[:, 0:512], wv_sb[:, c, :], xt_t,
                                 start=st_, stop=sp_)
            gs = slice(512 * g, 512 * (g + 1))
            nc.vector.tensor_scalar_add(qt_sb[:, gs], psqk[:, 0:512],
                                        qb_sb[:, 0:1])
            nc.vector.tensor_scalar_add(kt_sb[:, gs], psqk[:, 512:1024],
                                        kb_sb[:, 0:1])
            nc.vector.tensor_scalar_add(vt_sb[:, gs], psv[:, 0:512],
                                        vb_sb[:, 0:1])

            # transpose this group's V token blocks ([128 dims, 128 kpos]
            # -> [128 kpos, 128 dims], both heads at once) via DMA xbar, ALL
            # on the Sync queue: xbar transfers from different queues
            # intermittently corrupt each other, so they must be serialized
            # on a single queue.  A DVE copy then splits the two heads into
            # the ones-adjacent v65 layout.
            b = g // (NTT // BS)
            tstage = tsp.tile([128, 4, 128], DT, tag="ts", name="tstage")
            for j in range(4):
                ktl = 4 * (g % 4) + j
                t0 = QLEN * b + 128 * ktl
                nc.sync.dma_start(tstage[:, j, :], vt_sb[:, t0:t0 + 128],
                                  transpose=True)
            i0 = (4 * (g % 4)) * 2
            for h in range(2):
                nc.gpsimd.tensor_copy(
                    v65_sb[:, b, i0 + h:i0 + 8:2, 0:64],
                    tstage[:, :, 64 * h:64 * (h + 1)])

        # --- phase 2: attention + output projection per (batch, qtile) ---
        # Deferred-work queue: items emitted one per kt unit inside the next
        # qtile's loop so the divide / output projection / DMA issue work of
        # qtile i overlaps qtile i+1's scores+PV stream.
        from collections import deque
        pend_pe = deque()    # PE-work items (outproj): pop every other unit

        def emit_divide(cts, qs):
            """Denominators -> reciprocal -> normalized ctx in ct_sb.
            The sums-row copies run on ScalarE, which is naturally starved at
            qtile transitions; this keeps the DVE free to run the reciprocal
            as soon as the broadcast lands, releasing the work-pool slot."""
            srow0 = srp.tile([1, QTW], DT, tag="srow", name="srow0")
            srow1 = srp.tile([1, QTW], DT, tag="srow", name="srow1")
            nc.vector.tensor_copy(srow0[:], cts[0][64:65, :])
            nc.vector.tensor_copy(srow1[:], cts[1][64:65, :])
            sums_bc = work.tile([128, QTW], F32, tag="work", name="sums_bc")
            for j2 in range(2):
                js = slice(512 * j2, 512 * (j2 + 1))
                nc.tensor.matmul(sums_bc[:, js], onesA_sb[:],
                                 srow0[:, js], start=True, stop=False,
                                 skip_group_check=True)
                nc.tensor.matmul(sums_bc[:, js], onesB_sb[:],
                                 srow1[:, js], start=False, stop=True,
                                 skip_group_check=True)
            nc.vector.reciprocal_approx_fast(rc_sb[:], sums_bc[:])
            nc.vector.tensor_mul(ct_sb[0:64, qs], cts[0][0:64, :],
                                 rc_sb[0:64, :])
            nc.vector.tensor_mul(ct_sb[64:128, qs], cts[1][0:64, :],
                                 rc_sb[64:128, :])

        def defer_outproj(qs, t, evict_eng=None):
            def go():
                tok0 = qs.start + 128 * t
                o_ps = work.tile([128, 1024], F32, tag="work", name="o_ps")
                lhs = ct_sb[:, tok0:tok0 + 128]
                nc.tensor.matmul(o_ps[:, 0:512], lhs, wo_sb[:, 0:512],
                                 start=True, stop=True)
                nc.tensor.matmul(o_ps[:, 512:1024], lhs,
                                 wo_sb[:, 512:1024], start=True, stop=True)
                o_sb = evict.tile([128, 1024], DT, tag="osb", name="o_sb")
                if evict_eng is nc.scalar:
                    nc.scalar.activation(o_sb[:], o_ps[:],
                                         mybir.ActivationFunctionType.Copy)
                else:
                    nc.vector.tensor_copy(o_sb[:], o_ps[:])
                nc.sync.dma_start(out[tok0:tok0 + 128, :], o_sb[:])
            return go

        def attention_stream():
            """Yields once per attention unit so the caller can interleave
            the first few units with the tail of the QKV phase (carrying the
            PE p-state across the phase boundary).  PV accumulators are
            allocated lazily at the first PV so the accp ring stays free for
            the remaining QKV psv tiles."""
            prev_q = None
            for b in range(BS):
                for qt_i in range(NQT):
                    qs = slice(QLEN * b + QTW * qt_i,
                               QLEN * b + QTW * (qt_i + 1))
                    cts = []

                    def emit_pv(h, kt_i, st_t, cts=cts, b=b):
                        vsl = v65_sb[:, b, kt_i * 2 + h, 0:65]
                        st0, sp0 = (kt_i == 0), (kt_i == NKT - 1)
                        for j2 in range(2):
                            nc.tensor.matmul(
                                cts[h][0:65, 512 * j2:512 * (j2 + 1)],
                                vsl, st_t[:, 512 * j2:512 * (j2 + 1)],
                                start=st0, stop=sp0, skip_group_check=True)

                    # one continuous 32-unit stream across both heads: the
                    # lagged-PV pipeline crosses the head boundary with no
                    # drain gap.
                    lagged = []
                    for u in range(2 * NKT):
                        h, kt_i = u // NKT, u % NKT
                        hs = slice(HEAD * h, HEAD * (h + 1))
                        ks = slice(QLEN * b + 128 * kt_i,
                                   QLEN * b + 128 * (kt_i + 1))
                        s_ps = work.tile([128, QTW], F32, tag="work",
                                         name="s_ps")
                        for j2 in range(2):
                            qsub = slice(qs.start + 512 * j2,
                                         qs.start + 512 * (j2 + 1))
                            nc.tensor.matmul(
                                s_ps[:, 512 * j2:512 * (j2 + 1)],
                                kt_sb[hs, ks], qt_sb[hs, qsub],
                                start=True, stop=True)
                        st_t = stp.tile([128, QTW], DT, tag="st",
                                        name="st_t")
                        m_ap = mask_sb[:, b * NKT + kt_i:
                                       b * NKT + kt_i + 1]
                        nc.scalar.activation(
                            st_t[:], s_ps[:],
                            mybir.ActivationFunctionType.Exp, bias=m_ap)
                        lagged.append((h, kt_i, st_t))
                        # previous qtile's divide goes early so its outproj
                        # (and this qtile's first PV, via accp) unblock
                        if u == 1 and prev_q is not None:
                            emit_divide(*prev_q)
                            for t in range(QTW // 128):
                                pend_pe.append(defer_outproj(prev_q[1], t))
                            prev_q = None
                        if len(lagged) > 3:
                            if not cts:
                                cts.append(accp.tile([128, QTW], F32,
                                                     tag="acc", name="ct0"))
                                cts.append(accp.tile([128, QTW], F32,
                                                     tag="acc", name="ct1"))
                            emit_pv(*lagged.pop(0))
                        if u >= 3 and u % 4 == 3 and pend_pe:
                            pend_pe.popleft()()
                        yield
                    for item in lagged:
                        emit_pv(*item)
                    prev_q = (cts, qs)

            # final qtile drain (ScalarE is idle now: alternate evictors)
            emit_divide(*prev_q)
            for t in range(QTW // 128):
                defer_outproj(prev_q[1], t,
                              evict_eng=nc.scalar if t % 2 else nc.vector)()
            while pend_pe:
                pend_pe.popleft()()

        stream = attention_stream()
        for g in range(NTT):
            emit_group(g)
            # bridge the phase boundary: the first batch's attention units
            # (scores+exp only, no PV/psum pressure) slot between the last
            # QKV groups so the PE never idles at the transition.
            if 4 <= g < NTT - 1:
                next(stream, None)
        for _ in stream:
            pass

        if dump:
            nc.sync.dma_start(dbg["d_qt"][:], qt_sb[:])
            nc.sync.dma_start(dbg["d_kt"][:], kt_sb[:])
            nc.sync.dma_start(dbg["d_vt"][:], vt_sb[:])
            nc.sync.dma_start(
                dbg["d_v65"].rearrange("p (i c) -> p i c", c=65),
                v65_sb[:, :, :, 0:65].rearrange("p a b c -> p (a b) c"))
            nc.sync.dma_start(dbg["d_ct"][:], ct_sb[:])
            nc.sync.dma_start(dbg["d_rc"][:], rc_sb[:])

    nc.compile()
    return nc


def shard_inputs(input, mask, q_w, q_b, k_w, k_b, v_w, v_b, o_w, o_b):
    x = np.asarray(input, np.float32)
    xt = np.ascontiguousarray(x.T).astype(NPDT)
    m = np.asarray(mask, np.float32).reshape(BS, NKT, 128)
    maskd = np.ascontiguousarray(m.transpose(2, 0, 1).reshape(128, BS * NKT))
    scale = 1.0 / math.sqrt(HEAD)
    in_maps = []
    for c in range(NCORES):
        L = slice(LDIM * c, LDIM * (c + 1))
        in_maps.append({
            "xt": xt,
            "wq": np.ascontiguousarray((q_w[L, :] * scale).T).astype(NPDT),
            "wk": np.ascontiguousarray(k_w[L, :].T).astype(NPDT),
            "wv": np.ascontiguousarray(v_w[L, :].T).astype(NPDT),
            "wo": np.ascontiguousarray(o_w[:, L].T).astype(NPDT),
            "qb": (q_b[L] * scale).astype(np.float32).reshape(LDIM, 1),
            "kb": k_b[L].astype(np.float32).reshape(LDIM, 1),
            "vb": v_b[L].astype(np.float32).reshape(LDIM, 1),
            "maskd": maskd,
        })
    return in_maps


def run(in_maps, **kw):
    if "nc" not in _cache:
        _cache["nc"] = build_program()
    return run_bass_kernel_spmd(_cache["nc"], in_maps,
                                core_ids=list(range(NCORES)), **kw)


def kernel(input, mask, q_w, q_b, k_w, k_b, v_w, v_b, o_w, o_b,
           bs=BS, qlen=QLEN):
    assert int(bs) == BS and int(qlen) == QLEN
    in_maps = shard_inputs(np.asarray(input), np.asarray(mask),
                           np.asarray(q_w), np.asarray(q_b),
                           np.asarray(k_w), np.asarray(k_b),
                           np.asarray(v_w), np.asarray(v_b),
                           np.asarray(o_w), np.asarray(o_b))
    res = run(in_maps)
    acc = np.zeros((NTOK, DIM), np.float32)
    for r in res.results:
        acc += np.asarray(r["out"], np.float32)
    acc += np.asarray(o_b, np.float32)[None, :]
    return acc
